# revision 1
# baseline (speedup 1.0000x reference)
"""DbrxExpertGLU (single-expert SwiGLU MLP) Trainium2 kernel.

  down = (silu(x @ w1.T) * (x @ v1.T)) @ w2
  x: [4096, 4096] f32, w1/v1/w2: [14336, 4096] f32 -> out [4096, 4096] f32

Strategy (8 NeuronCores, tensor-parallel over ffn dim per the expert-TP
hint): shard F=14336 into 8 x 1792. Each core computes gate/up/inter for
its F-shard and a partial down [4096, 4096]; the host sums the 8 fp32
partials (cheaper than an on-device all-reduce and off the HW critical
path).

On-device layout is activation-transposed ([feature, token]) so all three
matmuls chain with weights stationary and no transposes:
  gateT[f,t] = sum_h w1[f,h] x[t,h];  upT likewise
  interT     = sigmoid(gateT)*gateT*upT  (ACT+DVE, cast to bf16)
  downT[h,t] = sum_f w2[f,h] interT[f,t]
Matmuls run in bf16 (fp32 PSUM accumulation) -> PE at 1 cycle/row; the
whole kernel is PE-bound at ~98% of the bf16 roofline (~2.3 ms/core).
Host pre-casts/pre-tiles inputs so every DMA lands >=1KB-contiguous per
partition.
"""

import time
from contextlib import ExitStack

import numpy as np
import ml_dtypes

import concourse.bass as bass
import concourse.mybir as mybir
import concourse.tile as tile
from concourse import bacc
from concourse.bass_utils import run_bass_kernel_spmd

BF16 = mybir.dt.bfloat16
F32 = mybir.dt.float32

T, H, F = 4096, 4096, 14336
N_CORES = 8
FS = F // N_CORES           # 1792 ffn rows per core
TC = 512                    # token chunk (= matmul moving dim)
NT, KB, FBN, HB = T // TC, H // 128, FS // 128, H // 128

_NC_CACHE = []


def _build():
    nc = bacc.Bacc("TRN2", target_bir_lowering=False, debug=False)

    xh = nc.dram_tensor("xh", [NT, KB, 128, TC], BF16, kind="ExternalInput").ap()
    w1h = nc.dram_tensor("w1h", [FBN, 128, KB, 128], BF16, kind="ExternalInput").ap()
    v1h = nc.dram_tensor("v1h", [FBN, 128, KB, 128], BF16, kind="ExternalInput").ap()
    w2h = nc.dram_tensor("w2h", [HB, 128, FBN, 128], BF16, kind="ExternalInput").ap()
    out = nc.dram_tensor("out", [H, T], F32, kind="ExternalOutput").ap()

    with tile.TileContext(nc) as tc, ExitStack() as ctx:
        xc_pool = ctx.enter_context(tc.tile_pool(name="xc", bufs=2))
        w1_pool = ctx.enter_context(tc.tile_pool(name="w1", bufs=3))
        v1_pool = ctx.enter_context(tc.tile_pool(name="v1", bufs=3))
        w2_pool = ctx.enter_context(tc.tile_pool(name="w2", bufs=3))
        inter_pool = ctx.enter_context(tc.tile_pool(name="inter", bufs=2))
        silu_pool = ctx.enter_context(tc.tile_pool(name="silu", bufs=3))
        out_pool = ctx.enter_context(tc.tile_pool(name="outp", bufs=4))
        pg_pool = ctx.enter_context(tc.tile_pool(name="pg", bufs=2, space="PSUM"))
        pu_pool = ctx.enter_context(tc.tile_pool(name="pu", bufs=2, space="PSUM"))
        pd_pool = ctx.enter_context(tc.tile_pool(name="pd", bufs=3, space="PSUM"))

        for tci in range(NT):
            # x chunk, free dim = (kb, t): rhs tiles for every h-block
            xc = xc_pool.tile([128, KB * TC], BF16)
            nc.sync.dma_start(
                out=xc[:].rearrange("p (kb t) -> p kb t", kb=KB),
                in_=xh[tci].rearrange("kb p t -> p kb t"),
            )
            inter = inter_pool.tile([128, FBN * TC], BF16)

            # phase A: gateT/upT -> interT, one f-block (128 rows) at a time
            for fb in range(FBN):
                w1f = w1_pool.tile([128, KB * 128], BF16)
                nc.sync.dma_start(
                    out=w1f[:].rearrange("p (kb f) -> p kb f", kb=KB), in_=w1h[fb]
                )
                v1f = v1_pool.tile([128, KB * 128], BF16)
                nc.sync.dma_start(
                    out=v1f[:].rearrange("p (kb f) -> p kb f", kb=KB), in_=v1h[fb]
                )
                pg = pg_pool.tile([128, TC], F32)
                pu = pu_pool.tile([128, TC], F32)
                for kb in range(KB):
                    nc.tensor.matmul(
                        pg[:], w1f[:, bass.ts(kb, 128)], xc[:, bass.ts(kb, TC)],
                        start=(kb == 0), stop=(kb == KB - 1),
                    )
                for kb in range(KB):
                    nc.tensor.matmul(
                        pu[:], v1f[:, bass.ts(kb, 128)], xc[:, bass.ts(kb, TC)],
                        start=(kb == 0), stop=(kb == KB - 1),
                    )
                sg = silu_pool.tile([128, TC], F32)
                nc.scalar.activation(
                    sg[:], pg[:], mybir.ActivationFunctionType.Sigmoid
                )
                sl = silu_pool.tile([128, TC], F32)
                nc.vector.tensor_mul(sl[:], sg[:], pg[:])
                nc.vector.tensor_mul(inter[:, bass.ts(fb, TC)], sl[:], pu[:])

            # phase B: partial downT, one h-block at a time
            for hb in range(HB):
                w2t = w2_pool.tile([128, FBN * 128], BF16)
                nc.sync.dma_start(
                    out=w2t[:].rearrange("p (fb h) -> p fb h", fb=FBN), in_=w2h[hb]
                )
                pd = pd_pool.tile([128, TC], F32)
                for fb in range(FBN):
                    nc.tensor.matmul(
                        pd[:], w2t[:, bass.ts(fb, 128)], inter[:, bass.ts(fb, TC)],
                        start=(fb == 0), stop=(fb == FBN - 1),
                    )
                ob = out_pool.tile([128, TC], F32)
                nc.scalar.copy(ob[:], pd[:])
                nc.sync.dma_start(
                    out=out[hb * 128:(hb + 1) * 128, bass.ts(tci, TC)], in_=ob[:]
                )

    nc.compile()
    return nc


def _prep_inputs(x, w1, v1, w2):
    bf = ml_dtypes.bfloat16
    # x[t, h] -> xh[tc, kb, p(h%128), tt]
    xh = np.ascontiguousarray(
        x.astype(bf).reshape(NT, TC, KB, 128).transpose(0, 2, 3, 1)
    )
    in_maps = []
    for c in range(N_CORES):
        sl = slice(c * FS, (c + 1) * FS)
        w1s = w1[sl].astype(bf)
        v1s = v1[sl].astype(bf)
        w2s = w2[sl].astype(bf)
        in_maps.append({
            "xh": xh,
            # w1[f, h] -> [fb, p(h%128), kb, ff]
            "w1h": np.ascontiguousarray(
                w1s.reshape(FBN, 128, KB, 128).transpose(0, 3, 2, 1)
            ),
            "v1h": np.ascontiguousarray(
                v1s.reshape(FBN, 128, KB, 128).transpose(0, 3, 2, 1)
            ),
            # w2[f, h] -> [hb, p(f%128), fb, hh]
            "w2h": np.ascontiguousarray(
                w2s.reshape(FBN, 128, HB, 128).transpose(2, 1, 0, 3)
            ),
        })
    return in_maps


def kernel(x, expert_w1, expert_v1, expert_w2):
    x = np.asarray(x, dtype=np.float32)
    expert_w1 = np.asarray(expert_w1, dtype=np.float32)
    expert_v1 = np.asarray(expert_v1, dtype=np.float32)
    expert_w2 = np.asarray(expert_w2, dtype=np.float32)
    assert x.shape == (T, H) and expert_w1.shape == (F, H)

    if not _NC_CACHE:
        _NC_CACHE.append(_build())
    nc = _NC_CACHE[0]
    in_maps = _prep_inputs(x, expert_w1, expert_v1, expert_w2)

    last_err = None
    for attempt in range(4):
        try:
            res = run_bass_kernel_spmd(nc, in_maps, list(range(N_CORES)))
            acc = res.results[0]["out"].astype(np.float32)
            for c in range(1, N_CORES):
                acc += res.results[c]["out"]
            if not np.isfinite(acc).all():
                raise FloatingPointError("non-finite output from device")
            return np.ascontiguousarray(acc.T)  # [h, t] -> [t, h]
        except Exception as e:  # transient device/tunnel errors: retry
            last_err = e
            time.sleep(3.0)
    raise last_err


# revision 5
# speedup vs baseline: 1.0059x; 1.0059x over previous
"""DbrxExpertGLU (single-expert SwiGLU MLP) Trainium2 kernel.

  down = (silu(x @ w1.T) * (x @ v1.T)) @ w2
  x: [4096, 4096] f32, w1/v1/w2: [14336, 4096] f32 -> out [4096, 4096] f32

Strategy (8 NeuronCores, tensor-parallel over ffn dim per the expert-TP
hint): shard F=14336 into 8 x 1792. Each core computes gate/up/inter for
its F-shard and a partial down [4096, 4096]; the host sums the 8 fp32
partials (cheaper than an on-device all-reduce and off the HW critical
path).

On-device layout is activation-transposed ([feature, token]) so all three
matmuls chain with weights stationary and no transposes:
  gateT[f,t] = sum_h w1[f,h] x[t,h];  upT likewise
  interT     = sigmoid(gateT)*gateT*upT  (ACT+DVE, cast to bf16)
  downT[h,t] = sum_f w2[f,h] interT[f,t]
Matmuls run in bf16 (fp32 PSUM accumulation) -> PE at 1 cycle/row; the
whole kernel is PE-bound at ~98% of the bf16 roofline (~2.3 ms/core).
Host pre-casts/pre-tiles inputs so every DMA lands >=1KB-contiguous per
partition.
"""

import os
import subprocess
import sys
import tempfile
import time
from contextlib import ExitStack

import numpy as np
import ml_dtypes

import concourse.bass as bass
import concourse.mybir as mybir
import concourse.tile as tile
from concourse import bacc
from concourse.bass_utils import run_bass_kernel_spmd

BF16 = mybir.dt.bfloat16
F32 = mybir.dt.float32

T, H, F = 4096, 4096, 14336
N_CORES = 8
FS = F // N_CORES           # 1792 ffn rows per core
TC = 512                    # token chunk (= matmul moving dim)
NT, KB, FBN, HB = T // TC, H // 128, FS // 128, H // 128

_NC_CACHE = []


def _build():
    nc = bacc.Bacc("TRN2", target_bir_lowering=False, debug=False)

    xh = nc.dram_tensor("xh", [NT, KB, 128, TC], BF16, kind="ExternalInput").ap()
    w1h = nc.dram_tensor("w1h", [FBN, 128, KB, 128], BF16, kind="ExternalInput").ap()
    v1h = nc.dram_tensor("v1h", [FBN, 128, KB, 128], BF16, kind="ExternalInput").ap()
    w2h = nc.dram_tensor("w2h", [HB, 128, FBN, 128], BF16, kind="ExternalInput").ap()
    out = nc.dram_tensor("out", [H, T], F32, kind="ExternalOutput").ap()

    with tile.TileContext(nc) as tc, ExitStack() as ctx:
        xc_pool = ctx.enter_context(tc.tile_pool(name="xc", bufs=2))
        w1_pool = ctx.enter_context(tc.tile_pool(name="w1", bufs=3))
        v1_pool = ctx.enter_context(tc.tile_pool(name="v1", bufs=3))
        w2_pool = ctx.enter_context(tc.tile_pool(name="w2", bufs=3))
        inter_pool = ctx.enter_context(tc.tile_pool(name="inter", bufs=2))
        silu_pool = ctx.enter_context(tc.tile_pool(name="silu", bufs=3))
        out_pool = ctx.enter_context(tc.tile_pool(name="outp", bufs=4))
        pg_pool = ctx.enter_context(tc.tile_pool(name="pg", bufs=2, space="PSUM"))
        pu_pool = ctx.enter_context(tc.tile_pool(name="pu", bufs=2, space="PSUM"))
        pd_pool = ctx.enter_context(tc.tile_pool(name="pd", bufs=3, space="PSUM"))

        for tci in range(NT):
            # x chunk, free dim = (kb, t): rhs tiles for every h-block
            xc = xc_pool.tile([128, KB * TC], BF16)
            if tci == 0:
                # fine-grained first load on the otherwise-idle ACT HWDGE
                # ring (parallel to weight DMAs on SP) so the PE starts on
                # kb=0 ~11us sooner instead of waiting for the whole 4MB
                for k0 in range(0, KB, 4):
                    nc.scalar.dma_start(
                        out=xc[:, k0 * TC:(k0 + 4) * TC].rearrange(
                            "p (kb t) -> p kb t", kb=4
                        ),
                        in_=xh[tci, k0:k0 + 4].rearrange("kb p t -> p kb t"),
                    )
            else:
                nc.sync.dma_start(
                    out=xc[:].rearrange("p (kb t) -> p kb t", kb=KB),
                    in_=xh[tci].rearrange("kb p t -> p kb t"),
                )
            inter = inter_pool.tile([128, FBN * TC], BF16)

            # phase A: gateT/upT -> interT, one f-block (128 rows) at a time
            for fb in range(FBN):
                w1f = w1_pool.tile([128, KB * 128], BF16)
                if tci == 0 and fb == 0:
                    for k0 in range(0, KB, 8):
                        nc.sync.dma_start(
                            out=w1f[:, k0 * 128:(k0 + 8) * 128].rearrange(
                                "p (kb f) -> p kb f", kb=8
                            ),
                            in_=w1h[fb][:, k0:k0 + 8],
                        )
                else:
                    nc.sync.dma_start(
                        out=w1f[:].rearrange("p (kb f) -> p kb f", kb=KB), in_=w1h[fb]
                    )
                v1f = v1_pool.tile([128, KB * 128], BF16)
                nc.sync.dma_start(
                    out=v1f[:].rearrange("p (kb f) -> p kb f", kb=KB), in_=v1h[fb]
                )
                pg = pg_pool.tile([128, TC], F32)
                pu = pu_pool.tile([128, TC], F32)
                for kb in range(KB):
                    nc.tensor.matmul(
                        pg[:], w1f[:, bass.ts(kb, 128)], xc[:, bass.ts(kb, TC)],
                        start=(kb == 0), stop=(kb == KB - 1),
                    )
                for kb in range(KB):
                    nc.tensor.matmul(
                        pu[:], v1f[:, bass.ts(kb, 128)], xc[:, bass.ts(kb, TC)],
                        start=(kb == 0), stop=(kb == KB - 1),
                    )
                sg = silu_pool.tile([128, TC], F32)
                nc.scalar.activation(
                    sg[:], pg[:], mybir.ActivationFunctionType.Sigmoid
                )
                sl = silu_pool.tile([128, TC], F32)
                nc.vector.tensor_mul(sl[:], sg[:], pg[:])
                nc.vector.tensor_mul(inter[:, bass.ts(fb, TC)], sl[:], pu[:])

            # phase B: partial downT, one h-block at a time
            for hb in range(HB):
                w2t = w2_pool.tile([128, FBN * 128], BF16)
                nc.sync.dma_start(
                    out=w2t[:].rearrange("p (fb h) -> p fb h", fb=FBN), in_=w2h[hb]
                )
                pd = pd_pool.tile([128, TC], F32)
                for fb in range(FBN):
                    nc.tensor.matmul(
                        pd[:], w2t[:, bass.ts(fb, 128)], inter[:, bass.ts(fb, TC)],
                        start=(fb == 0), stop=(fb == FBN - 1),
                    )
                ob = out_pool.tile([128, TC], F32)
                nc.scalar.copy(ob[:], pd[:])
                nc.sync.dma_start(
                    out=out[hb * 128:(hb + 1) * 128, bass.ts(tci, TC)], in_=ob[:]
                )

    nc.compile()
    return nc


def _prep_inputs(x, w1, v1, w2):
    bf = ml_dtypes.bfloat16
    # x[t, h] -> xh[tc, kb, p(h%128), tt]
    xh = np.ascontiguousarray(
        x.astype(bf).reshape(NT, TC, KB, 128).transpose(0, 2, 3, 1)
    )
    in_maps = []
    for c in range(N_CORES):
        sl = slice(c * FS, (c + 1) * FS)
        w1s = w1[sl].astype(bf)
        v1s = v1[sl].astype(bf)
        w2s = w2[sl].astype(bf)
        in_maps.append({
            "xh": xh,
            # w1[f, h] -> [fb, p(h%128), kb, ff]
            "w1h": np.ascontiguousarray(
                w1s.reshape(FBN, 128, KB, 128).transpose(0, 3, 2, 1)
            ),
            "v1h": np.ascontiguousarray(
                v1s.reshape(FBN, 128, KB, 128).transpose(0, 3, 2, 1)
            ),
            # w2[f, h] -> [hb, p(f%128), fb, hh]
            "w2h": np.ascontiguousarray(
                w2s.reshape(FBN, 128, HB, 128).transpose(2, 1, 0, 3)
            ),
        })
    return in_maps


def _exec_once(in_maps):
    """One 8-core device execution; returns summed partial [H, T] f32."""
    if not _NC_CACHE:
        _NC_CACHE.append(_build())
    res = run_bass_kernel_spmd(_NC_CACHE[0], in_maps, list(range(N_CORES)))
    acc = res.results[0]["out"].astype(np.float32)
    for c in range(1, N_CORES):
        acc += res.results[c]["out"]
    if not np.isfinite(acc).all():
        raise FloatingPointError("non-finite output from device")
    return acc


def _exec_subprocess(in_maps):
    """Retry path: run the device execution in a fresh process (fresh axon
    client) in case this process's device session is poisoned."""
    base = "/dev/shm" if os.path.isdir("/dev/shm") else None
    with tempfile.TemporaryDirectory(dir=base) as d:
        np.save(os.path.join(d, "xh.npy"), in_maps[0]["xh"].view(np.uint16))
        for c, m in enumerate(in_maps):
            for k in ("w1h", "v1h", "w2h"):
                np.save(os.path.join(d, f"{k}_{c}.npy"), m[k].view(np.uint16))
        subprocess.run(
            [sys.executable, os.path.abspath(__file__), "--subproc", d],
            check=True, timeout=1200,
        )
        return np.load(os.path.join(d, "acc.npy"))


def _subproc_main(d):
    bf = ml_dtypes.bfloat16
    xh = np.load(os.path.join(d, "xh.npy")).view(bf)
    in_maps = []
    for c in range(N_CORES):
        m = {"xh": xh}
        for k in ("w1h", "v1h", "w2h"):
            m[k] = np.load(os.path.join(d, f"{k}_{c}.npy")).view(bf)
        in_maps.append(m)
    np.save(os.path.join(d, "acc.npy"), _exec_once(in_maps))


def kernel(x, expert_w1, expert_v1, expert_w2):
    x = np.asarray(x, dtype=np.float32)
    expert_w1 = np.asarray(expert_w1, dtype=np.float32)
    expert_v1 = np.asarray(expert_v1, dtype=np.float32)
    expert_w2 = np.asarray(expert_w2, dtype=np.float32)
    assert x.shape == (T, H) and expert_w1.shape == (F, H)

    in_maps = _prep_inputs(x, expert_w1, expert_v1, expert_w2)

    acc = None
    last_err = None
    for attempt in range(4):
        try:
            if attempt < 2:
                acc = _exec_once(in_maps)
            else:
                acc = _exec_subprocess(in_maps)
            break
        except Exception as e:  # transient device/tunnel errors: retry
            last_err = e
            time.sleep(3.0)
    if acc is None:
        raise last_err
    return np.ascontiguousarray(acc.T)  # [h, t] -> [t, h]


if __name__ == "__main__" and len(sys.argv) == 3 and sys.argv[1] == "--subproc":
    _subproc_main(sys.argv[2])
